# revision 6
# baseline (speedup 1.0000x reference)
"""H2O Llama attention (streaming) — Trainium2 Bass kernel, 8-core head-parallel.

Sharding: 32 q-heads / 8 kv-heads split across 8 cores (4 q-heads + 1 kv-head
per core). Each core computes its heads' QKV projections, RoPE, causal
attention with H2O score accumulation, and a partial output projection
(attn_out partial summed over cores on host). Top-k heavy-hitter selection +
KV gather runs on host from device-computed hh scores / rope'd K / V.
"""
import sys
sys.path.insert(0, "/opt/trn_rl_repo")
import numpy as np

P = 128
S = 2048
D = 4096
NH = 4          # local q heads per core
HID = 128
NCHUNK = S // P   # 16
MASK = -30000.0

_CACHE = {}
LAST_RESULT = None


def _build_bass():
    import concourse.bass as bass
    from concourse import bacc
    import concourse.mybir as mybir
    import concourse.tile as tile
    from concourse.masks import make_identity
    from contextlib import ExitStack

    f32 = mybir.dt.float32
    X = mybir.AxisListType.X
    Exp = mybir.ActivationFunctionType.Exp

    nc = bacc.Bacc("TRN2", target_bir_lowering=False)
    hT = nc.dram_tensor("hT", [D, S], f32, kind="ExternalInput")
    wq = nc.dram_tensor("wq", [D, NH * HID], f32, kind="ExternalInput")
    wk = nc.dram_tensor("wk", [D, HID], f32, kind="ExternalInput")
    wv = nc.dram_tensor("wv", [D, HID], f32, kind="ExternalInput")
    wo = nc.dram_tensor("wo", [NH * HID, D], f32, kind="ExternalInput")
    cosq = nc.dram_tensor("cosq", [HID, S], f32, kind="ExternalInput")
    sinq = nc.dram_tensor("sinq", [HID, S], f32, kind="ExternalInput")
    cosk = nc.dram_tensor("cosk", [HID, S], f32, kind="ExternalInput")
    sink = nc.dram_tensor("sink", [HID, S], f32, kind="ExternalInput")
    tri = nc.dram_tensor("tri", [P, P], f32, kind="ExternalInput")

    attn_part = nc.dram_tensor("attn_part", [S, D], f32, kind="ExternalOutput")
    hh_out = nc.dram_tensor("hh_out", [NH, S], f32, kind="ExternalOutput")
    k_out = nc.dram_tensor("k_out", [S, HID], f32, kind="ExternalOutput")
    v_out = nc.dram_tensor("v_out", [S, HID], f32, kind="ExternalOutput")

    qT_s = nc.dram_tensor("qT_s", [NH * HID, S], f32, kind="Internal")
    kT_s = nc.dram_tensor("kT_s", [HID, S], f32, kind="Internal")
    vT_s = nc.dram_tensor("vT_s", [HID, S], f32, kind="Internal")

    NS = 256  # seq cols per projection slice
    with tile.TileContext(nc) as tc:
        with ExitStack() as octx:
            const = octx.enter_context(tc.tile_pool(name="const", bufs=1))
            ident = const.tile([P, P], f32)
            make_identity(nc, ident[:])
            tri_sb = const.tile([P, P], f32)
            nc.gpsimd.dma_start(out=tri_sb[:], in_=tri[:])

            # ---------------- Phase A: projections (d-major outputs) --------
            with ExitStack() as ctx:
                wp = ctx.enter_context(tc.tile_pool(name="wp", bufs=1))
                hp = ctx.enter_context(tc.tile_pool(name="hp", bufs=2))
                ep = ctx.enter_context(tc.tile_pool(name="ep", bufs=3))
                pp = ctx.enter_context(tc.tile_pool(name="pp", bufs=2, space="PSUM"))
                wq_sb = wp.tile([P, 32 * 512], f32)
                wk_sb = wp.tile([P, 32 * 128], f32)
                wv_sb = wp.tile([P, 32 * 128], f32)
                nc.gpsimd.dma_start(out=wq_sb[:].rearrange("p (c m) -> p c m", c=32), in_=wq.rearrange("(c p) m -> p c m", p=P))
                nc.gpsimd.dma_start(out=wk_sb[:].rearrange("p (c m) -> p c m", c=32), in_=wk.rearrange("(c p) m -> p c m", p=P))
                nc.gpsimd.dma_start(out=wv_sb[:].rearrange("p (c m) -> p c m", c=32), in_=wv.rearrange("(c p) m -> p c m", p=P))
                targets = [(wq_sb, 512, h, qT_s, h * HID) for h in range(NH)]
                targets += [(wk_sb, 128, 0, kT_s, 0), (wv_sb, 128, 0, vT_s, 0)]
                for n in range(S // NS):
                    hs = hp.tile([P, 32 * NS], f32, tag="hs")
                    nc.gpsimd.dma_start(
                        out=hs[:].rearrange("p (c s) -> p c s", c=32),
                        in_=hT[:, n * NS:(n + 1) * NS].rearrange("(c p) s -> p c s", p=P),
                    )
                    for (wsb, wstride, m, dst, drow) in targets:
                        ps = pp.tile([P, NS], f32, tag="pj")
                        for ec in range(32):
                            nc.tensor.matmul(
                                out=ps[:],
                                lhsT=wsb[:, ec * wstride + m * HID: ec * wstride + (m + 1) * HID],
                                rhs=hs[:, ec * NS:(ec + 1) * NS],
                                start=(ec == 0), stop=(ec == 31),
                            )
                        ot = ep.tile([P, NS], f32, tag="pe")
                        nc.scalar.copy(ot[:], ps[:])
                        nc.gpsimd.dma_start(
                            out=dst[drow:drow + HID, n * NS:(n + 1) * NS], in_=ot[:]
                        )

            # ---------------- Phase B/C/D pools ----------------------------
            with ExitStack() as ctx:
                kv = ctx.enter_context(tc.tile_pool(name="kv", bufs=1))
                rp = ctx.enter_context(tc.tile_pool(name="rp", bufs=2))
                wb = ctx.enter_context(tc.tile_pool(name="wb", bufs=2))
                sp = ctx.enter_context(tc.tile_pool(name="sp", bufs=3))
                wt = ctx.enter_context(tc.tile_pool(name="wt", bufs=3))
                store = ctx.enter_context(tc.tile_pool(name="store", bufs=1))
                psc = ctx.enter_context(tc.tile_pool(name="psc", bufs=2, space="PSUM"))
                pst = ctx.enter_context(tc.tile_pool(name="pst", bufs=2, space="PSUM"))
                psa = ctx.enter_context(tc.tile_pool(name="psa", bufs=1, space="PSUM"))
                pso = ctx.enter_context(tc.tile_pool(name="pso", bufs=2, space="PSUM"))

                cq = kv.tile([HID, S], f32)
                sq = kv.tile([HID, S], f32)
                ck = kv.tile([HID, S], f32)
                sk = kv.tile([HID, S], f32)
                nc.gpsimd.dma_start(out=cq[:], in_=cosq[:])
                nc.gpsimd.dma_start(out=sq[:], in_=sinq[:])
                nc.gpsimd.dma_start(out=ck[:], in_=cosk[:])
                nc.gpsimd.dma_start(out=sk[:], in_=sink[:])

                def rope(dst, src_dram, row0, cos_sb, sin_sb):
                    raw = rp.tile([HID, S], f32, tag="raw")
                    nc.gpsimd.dma_start(out=raw[:], in_=src_dram[row0:row0 + HID, :])
                    shf = rp.tile([HID, S], f32, tag="shf")
                    nc.gpsimd.dma_start(out=shf[:64, :], in_=raw[64:128, :])
                    nc.gpsimd.dma_start(out=shf[64:128, :], in_=raw[:64, :])
                    nc.vector.tensor_mul(out=shf[:], in0=shf[:], in1=sin_sb[:])
                    nc.vector.tensor_mul(out=raw[:], in0=raw[:], in1=cos_sb[:])
                    nc.vector.tensor_add(out=dst[:], in0=raw[:], in1=shf[:])

                # K rope (unscaled) -> k2 resident; also emit s-major k to k_out
                k2 = kv.tile([HID, S], f32)
                rope(k2, kT_s, 0, ck, sk)
                v_sm = kv.tile([P, S], f32)   # chunk-major [key-in-chunk, chunk*128+d]
                vT_sb = kv.tile([HID, S], f32)
                nc.gpsimd.dma_start(out=vT_sb[:], in_=vT_s[:])
                for c in range(NCHUNK):
                    tp = pst.tile([P, P], f32, tag="tp")
                    nc.tensor.transpose(tp[:], k2[:, c * P:(c + 1) * P], ident[:])
                    ko = sp.tile([P, P], f32, tag="ko")
                    nc.scalar.copy(ko[:], tp[:])
                    nc.sync.dma_start(out=k_out[c * P:(c + 1) * P, :], in_=ko[:])
                    tp2 = pst.tile([P, P], f32, tag="tp")
                    nc.tensor.transpose(tp2[:], vT_sb[:, c * P:(c + 1) * P], ident[:])
                    nc.vector.tensor_copy(out=v_sm[:, c * P:(c + 1) * P], in_=tp2[:])
                    nc.sync.dma_start(out=v_out[c * P:(c + 1) * P, :], in_=v_sm[:, c * P:(c + 1) * P])

                attnT_store = store.tile([P, NH * NCHUNK * P], f32)

                for h in range(NH):
                    q2 = rp.tile([HID, S], f32, tag="q2")
                    rope(q2, qT_s, h * HID, cq, sq)
                    hh_parts = wb.tile([P, NCHUNK * NCHUNK], f32, tag="hhp")
                    nc.vector.memset(hh_parts[:], 0.0)
                    for qb in range(NCHUNK):
                        K = (qb + 1) * P
                        nsl = (K + 511) // 512
                        w_sb = wb.tile([P, S], f32, tag="w")
                        parts = sp.tile([P, 4], f32, tag="parts")
                        for sl in range(nsl):
                            cw = min(512, K - sl * 512)
                            ps = psc.tile([P, 512], f32, tag="sc")
                            nc.tensor.matmul(
                                out=ps[:, :cw],
                                lhsT=q2[:, qb * P:(qb + 1) * P],
                                rhs=k2[:, sl * 512: sl * 512 + cw],
                                start=True, stop=True,
                            )
                            if sl == nsl - 1:  # diagonal chunk lives in last slice
                                off = qb * P - sl * 512
                                nc.vector.tensor_add(
                                    out=ps[:, off:off + P], in0=ps[:, off:off + P], in1=tri_sb[:]
                                )
                            nc.scalar.activation(
                                w_sb[:, sl * 512: sl * 512 + cw], ps[:, :cw], Exp,
                                accum_out=parts[:, sl:sl + 1],
                            )
                        sums = sp.tile([P, 1], f32, tag="sums")
                        nc.vector.reduce_sum(sums[:], parts[:, :nsl], axis=X)
                        recip = sp.tile([P, 1], f32, tag="recip")
                        nc.vector.reciprocal(recip[:], sums[:])
                        nc.scalar.mul(w_sb[:, :K], w_sb[:, :K], recip[:])
                        at_ps = psa.tile([P, P], f32, tag="at")
                        for c in range(qb + 1):
                            tp = pst.tile([P, P], f32, tag="tp")
                            nc.tensor.transpose(tp[:], w_sb[:, c * P:(c + 1) * P], ident[:])
                            wTt = wt.tile([P, P], f32, tag="wT")
                            nc.scalar.activation(
                                wTt[:], tp[:], mybir.ActivationFunctionType.Copy,
                                accum_out=hh_parts[:, qb * NCHUNK + c: qb * NCHUNK + c + 1],
                            )
                            nc.tensor.matmul(
                                out=at_ps[:], lhsT=v_sm[:, c * P:(c + 1) * P], rhs=wTt[:],
                                start=(c == 0), stop=(c == qb),
                            )
                        nc.vector.tensor_copy(
                            out=attnT_store[:, (h * NCHUNK + qb) * P:(h * NCHUNK + qb + 1) * P],
                            in_=at_ps[:],
                        )
                    hhk = sp.tile([P, NCHUNK], f32, tag="hhk")
                    nc.vector.reduce_sum(
                        hhk[:], hh_parts[:].rearrange("p (q c) -> p c q", q=NCHUNK), axis=X
                    )
                    nc.sync.dma_start(
                        out=hh_out[h, :].rearrange("(c k) -> k c", c=NCHUNK), in_=hhk[:]
                    )

                # ---------------- Phase D: output projection ----------------
                op = ctx.enter_context(tc.tile_pool(name="op", bufs=3))
                wop = ctx.enter_context(tc.tile_pool(name="wop", bufs=2))
                for n in range(D // 512):
                    wo_n = wop.tile([P, NH * 512], f32, tag="wo")
                    nc.gpsimd.dma_start(
                        out=wo_n[:].rearrange("p (h e) -> p h e", h=NH),
                        in_=wo[:, n * 512:(n + 1) * 512].rearrange("(h p) e -> p h e", p=P),
                    )
                    for qb in range(NCHUNK):
                        ps = pso.tile([P, 512], f32, tag="op")
                        for h in range(NH):
                            nc.tensor.matmul(
                                out=ps[:],
                                lhsT=attnT_store[:, (h * NCHUNK + qb) * P:(h * NCHUNK + qb + 1) * P],
                                rhs=wo_n[:, h * 512:(h + 1) * 512],
                                start=(h == 0), stop=(h == NH - 1),
                            )
                        ot = op.tile([P, 512], f32, tag="oe")
                        nc.scalar.copy(ot[:], ps[:])
                        nc.sync.dma_start(
                            out=attn_part[qb * P:(qb + 1) * P, n * 512:(n + 1) * 512], in_=ot[:]
                        )
    nc.compile()
    return nc


def _rope_consts():
    hid = HID
    inv_freq = 1.0 / (10000.0 ** (np.arange(0, hid, 2, dtype=np.float64) / hid))
    pos = np.arange(S, dtype=np.float64)
    freqs = pos[:, None] * inv_freq[None, :]
    emb = np.concatenate([freqs, freqs], axis=-1)          # [S, HID]
    cos = np.cos(emb).astype(np.float32).T.copy()          # [HID, S]
    sin = np.sin(emb).astype(np.float32).T.copy()
    sin_signed = sin.copy()
    sin_signed[:64] *= -1.0
    s2 = np.float32(1.0 / np.sqrt(128.0))
    return (np.ascontiguousarray(cos * s2), np.ascontiguousarray(sin_signed * s2),
            cos, sin_signed)


def kernel(hidden_states, Wq, Wk, Wv, Wo):
    global LAST_RESULT
    from concourse.bass_utils import run_bass_kernel_spmd

    if "nc" not in _CACHE:
        _CACHE["nc"] = _build_bass()
    nc = _CACHE["nc"]

    hT = np.ascontiguousarray(hidden_states[0].T.astype(np.float32))  # [D, S]
    cosq, sinq, cosk, sink = _rope_consts()
    tri = np.triu(np.full((P, P), MASK, dtype=np.float32), k=1)

    in_maps = []
    for c in range(8):
        in_maps.append({
            "hT": hT,
            "wq": np.ascontiguousarray(Wq[c * 512:(c + 1) * 512].T),
            "wk": np.ascontiguousarray(Wk[c * 128:(c + 1) * 128].T),
            "wv": np.ascontiguousarray(Wv[c * 128:(c + 1) * 128].T),
            "wo": np.ascontiguousarray(Wo[:, c * 512:(c + 1) * 512].T),
            "cosq": cosq, "sinq": sinq, "cosk": cosk, "sink": sink,
            "tri": tri,
        })

    res = run_bass_kernel_spmd(nc, in_maps, core_ids=list(range(8)))
    LAST_RESULT = res

    attn = np.zeros((S, D), dtype=np.float64)
    hh = np.zeros((32, S), dtype=np.float32)
    k_hh = np.zeros((32, 768, HID), dtype=np.float32)
    v_hh = np.zeros((32, 768, HID), dtype=np.float32)
    for c in range(8):
        r = res.results[c]
        attn += r["attn_part"].astype(np.float64)
        hh[c * 4:(c + 1) * 4] = r["hh_out"]
        kc, vc = r["k_out"], r["v_out"]
        for j in range(NH):
            g = c * 4 + j
            sel = hh[g, :S - 512]
            order = np.argsort(-sel, kind="stable")[:256]
            keep = np.sort(order)
            k_hh[g, :256] = kc[keep]
            v_hh[g, :256] = vc[keep]
            k_hh[g, 256:] = kc[S - 512:]
            v_hh[g, 256:] = vc[S - 512:]
    attn_out = attn.astype(np.float32)[None]        # [1, S, D]
    return attn_out, hh, k_hh[None], v_hh[None]
